# revision 1
# baseline (speedup 1.0000x reference)
"""Raw-Bacc v3: quarter-grained pipeline, DVE-only v computation,
loads split across both HWDGE queues, consts packed into one DMA.

out[n, c] = pf[c, n] + (Wv @ age + bv)[c]

wvx host-packed [128, 129]: cols 0:64 = Wv, 64:128 = age replicated to
every row, 128 = bv. v = reduce_sum(Wv * age_bc, free) + bv on VectorE
(no PE round-trip for the v chain).
"""

import numpy as np

N_CORES = 8
B, C, D, H, W = 1, 128, 16, 32, 32
N = D * H * W
NSH = N // N_CORES       # 2048
AGE = 64
QTR = 512                # quarter width
NQ = NSH // QTR          # 4


def build_nc():
    import concourse.bacc as bacc
    import concourse.mybir as mybir
    from contextlib import ExitStack

    f32 = mybir.dt.float32
    nc = bacc.Bacc(
        "TRN2", target_bir_lowering=False, debug=False, num_devices=N_CORES)
    pf = nc.dram_tensor("pf", [C, NSH], f32, kind="ExternalInput")
    wvx = nc.dram_tensor("wvx", [C, 2 * AGE + 1], f32, kind="ExternalInput")
    iden = nc.dram_tensor("iden", [128, 128], f32, kind="ExternalInput")
    out = nc.dram_tensor("out", [NSH, C], f32, kind="ExternalOutput")

    # out rows grouped [half h][quarter-in-half g][block j][partition p]
    outv = out.rearrange("(h g j p) c -> h p g j c", p=128, j=QTR // 128,
                         g=2)

    with ExitStack() as ctx:
        e = ctx.enter_context
        sid = e(nc.semaphore("sid"))
        swx = e(nc.semaphore("swx"))
        spf = [e(nc.semaphore(f"spf{q}")) for q in range(NQ)]
        sout = e(nc.semaphore("sout"))
        spe = e(nc.semaphore("spe"))
        sact = e(nc.semaphore("sact"))
        sv = e(nc.semaphore("sv"))
        svc = e(nc.semaphore("svc"))
        identsb = e(nc.sbuf_tensor("identsb", [128, 128], f32))
        wvxsb = e(nc.sbuf_tensor("wvxsb", [C, 2 * AGE + 1], f32))
        tmp = e(nc.sbuf_tensor("tmp", [C, AGE], f32))
        vsum = e(nc.sbuf_tensor("vsum", [C, 1], f32))
        vcol = e(nc.sbuf_tensor("vcol", [C, 1], f32))
        pft = e(nc.sbuf_tensor("pft", [C, NSH], f32))
        osb0 = e(nc.sbuf_tensor("osb0", [128, 2 * QTR], f32))
        osb1 = e(nc.sbuf_tensor("osb1", [128, 2 * QTR], f32))
        pgs = [e(nc.psum_tensor(f"pg{q}", [128, QTR], f32)) for q in range(NQ)]
        block = e(nc.Block())
        osbs = [osb0, osb1]

        @block.sync
        def _(sync):
            sync.dma_start(out=identsb[:], in_=iden[:]).then_inc(sid, 16)
            sync.dma_start(
                out=pft[:, 0 * QTR:1 * QTR],
                in_=pf[:, 0 * QTR:1 * QTR]).then_inc(spf[0], 16)
            sync.dma_start(
                out=pft[:, 2 * QTR:3 * QTR],
                in_=pf[:, 2 * QTR:3 * QTR]).then_inc(spf[2], 16)
            sync.wait_ge(svc, 2)
            sync.dma_start(
                out=outv[0],
                in_=osb0[:].rearrange("p (g j c) -> p g j c", c=128,
                                      j=QTR // 128),
            ).then_inc(sout, 16)
            sync.wait_ge(sout, 32)

        @block.scalar
        def _(scalar):
            import concourse.mybir as mybir

            scalar.dma_start(out=wvxsb[:], in_=wvx[:]).then_inc(swx, 16)
            scalar.dma_start(
                out=pft[:, 1 * QTR:2 * QTR],
                in_=pf[:, 1 * QTR:2 * QTR]).then_inc(spf[1], 16)
            scalar.dma_start(
                out=pft[:, 3 * QTR:4 * QTR],
                in_=pf[:, 3 * QTR:4 * QTR]).then_inc(spf[3], 16)
            scalar.wait_ge(sv, 1)
            for q in range(NQ):
                scalar.wait_ge(spf[q], 16)
                scalar.activation(
                    pft[:, q * QTR:(q + 1) * QTR],
                    pft[:, q * QTR:(q + 1) * QTR],
                    mybir.ActivationFunctionType.Identity,
                    bias=vcol[:],
                ).then_inc(sact, 1)
            scalar.wait_ge(svc, 4)
            scalar.dma_start(
                out=outv[1],
                in_=osb1[:].rearrange("p (g j c) -> p g j c", c=128,
                                      j=QTR // 128),
            ).then_inc(sout, 16)

        @block.tensor
        def _(tensor):
            tensor.wait_ge(sid, 16)
            for q in range(NQ):
                tensor.wait_ge(sact, q + 1)
                for j in range(QTR // 128):
                    c0 = q * QTR + j * 128
                    ins = tensor.transpose(
                        pgs[q][:, j * 128:(j + 1) * 128],
                        pft[:, c0:c0 + 128],
                        identsb[:],
                    )
                    if j == QTR // 128 - 1:
                        ins.then_inc(spe, 1)

        @block.vector
        def _(vector):
            import concourse.mybir as mybir

            vector.wait_ge(swx, 16)
            vector.tensor_tensor(
                tmp[:], wvxsb[:, 0:AGE], wvxsb[:, AGE:2 * AGE],
                mybir.AluOpType.mult)
            vector.reduce_sum(vsum[:], tmp[:], axis=mybir.AxisListType.X)
            vector.tensor_scalar(
                out=vcol[:], in0=vsum[:],
                scalar1=wvxsb[:, 2 * AGE:2 * AGE + 1], scalar2=None,
                op0=mybir.AluOpType.add,
            ).then_inc(sv, 1)
            for q in range(NQ):
                vector.wait_ge(spe, q + 1)
                vector.tensor_copy(
                    osbs[q // 2][:, (q % 2) * QTR:(q % 2 + 1) * QTR],
                    pgs[q][:],
                ).then_inc(svc, 1)

    nc.finalize()
    return nc


_CACHE = {}
LAST_RESULTS = None


def kernel(**inputs):
    global LAST_RESULTS
    from concourse.bass_utils import run_bass_kernel_spmd

    if "nc" not in _CACHE:
        _CACHE["nc"] = build_nc()
    nc = _CACHE["nc"]

    pf_full = np.ascontiguousarray(
        np.asarray(inputs["pixel_features"], dtype=np.float32).reshape(C, N))
    age = np.asarray(inputs["age_features"], dtype=np.float32).reshape(AGE)
    wvx_np = np.empty((C, 2 * AGE + 1), dtype=np.float32)
    wvx_np[:, 0:AGE] = np.asarray(inputs["Wv"], dtype=np.float32)
    wvx_np[:, AGE:2 * AGE] = age[None, :]
    wvx_np[:, 2 * AGE] = np.asarray(inputs["bv"], dtype=np.float32)
    iden_np = np.eye(128, dtype=np.float32)

    in_maps = [
        {
            "pf": np.ascontiguousarray(pf_full[:, i * NSH:(i + 1) * NSH]),
            "wvx": wvx_np,
            "iden": iden_np,
        }
        for i in range(N_CORES)
    ]
    res = run_bass_kernel_spmd(nc, in_maps, core_ids=list(range(N_CORES)))
    LAST_RESULTS = res
    out = np.concatenate([res.results[i]["out"] for i in range(N_CORES)], axis=0)
    return out.reshape(B, N, C).astype(np.float32)

